# revision 1
# baseline (speedup 1.0000x reference)
"""Self-contained Trainium2 Bass kernel for the sparse point-attention module.

Strategy: shard the point dimension n across the 8 NeuronCores (512 points
each, both batch entries on every core).  Each core gets the full `pos`
(tiny) so the KNN is purely local; everything else is data-parallel and no
collectives are needed.

v2 — engine-balanced rewrite of the baseline (868us):
  - KNN: centered distances dneg ~= -d via one 13-row hi/lo bf16 matmul
    (|p_i|^2 and |p_j|^2 both subtracted inside the PE, so the top values
    sit near 0).  The ACT engine evicts them as fp16 into the HIGH halves
    of a persistent u32 array whose LOW halves hold a one-time iota16; the
    u32 read as fp32 orders by (distance, index) with unique values, so a
    single max8 stream gives values AND indices.  Two-level scan: 8 grouped
    max8 over 512-point groups -> 64 candidates, then a tiny top-16 on the
    candidates (match_replace never touches the big array, so the iota
    lows stay pristine).  ~4x less DVE time than the 5-pass fp32 scan,
    and the scan for tile i+2 is pipelined 2 tiles ahead so the PE never
    drains (HAM stays warm: cold-clock was ~43% of the baseline run).
  - q-conv eliminated: pe1 = relu((Wp1f@Wq)·pos_g + b) comes straight from
    the 4-row gathered positions; rr = q - k_f + 1 is built by accumulating
    the Wq and -Wk matmuls into one PSUM tile; bp2 rides a ones-row of the
    pe1 activation tile; ba2 dropped (softmax-invariant), bv folded into
    the output bias (softmax weights sum to 1).
  - Eviction/elementwise engine balance: ACT does relu/exp/copy evictions,
    DVE does the PSUM-operand products and 3 of 8 a1r evictions
    (tensor_scalar add-bias+relu), gpsimd does the softmax tail
    (esum/e*vpe/aggc) plus gathers and k/v DMA prefetch (2 chunks ahead).
    bf16 intermediates give 2x DVE modes where all operands are 2-byte.
"""

import numpy as np
import ml_dtypes

BF16 = ml_dtypes.bfloat16

# ---- problem dimensions (hardcoded, must match the grader's inputs) ----
B = 2
CIN = 128
N = 4096
KK = 16          # neighbours
DIM = 256
PHID = 64
AHID = 1024
NCORES = 8
NLOC = N // NCORES
BN_EPS = 1e-5
NEG_BIG = -1e30


def _dims_full():
    return dict(B=B, CIN=CIN, N=N, KK=KK, DIM=DIM, PHID=PHID, AHID=AHID,
                NLOC=NLOC)


def build_nc(dims):
    """Build the (single, SPMD) Bass program for one core's shard."""
    import concourse.bass as bass
    import concourse.mybir as mybir
    import concourse.tile as tile
    from concourse import bacc
    from concourse.bass import ts

    fp32 = mybir.dt.float32
    bf16 = mybir.dt.bfloat16
    fp16 = mybir.dt.float16
    u16 = mybir.dt.uint16
    i16 = mybir.dt.int16
    u32 = mybir.dt.uint32
    AF = mybir.ActivationFunctionType
    OP = mybir.AluOpType
    AX = mybir.AxisListType

    Bn = dims["B"]; CINn = dims["CIN"]; Nn = dims["N"]; KKn = dims["KK"]
    DIMn = dims["DIM"]; PHIDn = dims["PHID"]; AHIDn = dims["AHID"]
    NLOCn = dims["NLOC"]

    QT = min(128, NLOCn)              # queries per KNN tile
    NQT = NLOCn // QT                 # KNN tiles per batch
    CHUNK = 512                       # matmul column chunk (n,k cols)
    CQ = CHUNK // KKn                 # queries per chunk (32)
    NCH_TILE = (QT * KKn) // CHUNK    # chunks per KNN tile
    NCH_D = Nn // 512                 # 512-col chunks of the distance row
    DM = DIMn // 128                  # feature tiles (2)
    AM = AHIDn // 128                 # a-hidden tiles (8)
    KA1 = DIMn // 128                 # contraction tiles for a1 (2)
    GRP = 512                         # KNN L1 group size
    NGRP = Nn // GRP                  # 8 for the full problem
    TWO_LEVEL = NGRP >= 4             # direct scan for the small sim config
    PF = 2                            # k/v DMA prefetch depth (chunks)

    nc = bacc.Bacc()

    # ---- DRAM parameters ----
    key_r = nc.declare_dram_parameter("key_r", [Bn, CINn, NLOCn * KKn], bf16, isOutput=False)
    val_r = nc.declare_dram_parameter("val_r", [Bn, CINn, NLOCn * KKn], bf16, isOutput=False)
    paug_lhs = nc.declare_dram_parameter("paug_lhs", [Bn, 13, NLOCn], bf16, isOutput=False)
    paug_rhs = nc.declare_dram_parameter("paug_rhs", [Bn, 13, Nn], bf16, isOutput=False)
    pos16_d = nc.declare_dram_parameter("pos16", [Bn, 16, Nn], fp32, isOutput=False)
    iota_d = nc.declare_dram_parameter("iota32", [QT, Nn], u32, isOutput=False)
    WkTn_d = nc.declare_dram_parameter("WkTn", [CINn, DIMn], bf16, isOutput=False)
    WvT_d = nc.declare_dram_parameter("WvT", [CINn, DIMn], bf16, isOutput=False)
    WqTb_d = nc.declare_dram_parameter("WqTb", [4, DIMn], bf16, isOutput=False)
    Wp1q_d = nc.declare_dram_parameter("Wp1q", [4, PHIDn], bf16, isOutput=False)
    Wp2T_d = nc.declare_dram_parameter("Wp2T", [PHIDn + 1, DIMn], bf16, isOutput=False)
    Wa1T_d = nc.declare_dram_parameter("Wa1T", [128, KA1, AHIDn], bf16, isOutput=False)
    Wa2T_d = nc.declare_dram_parameter("Wa2T", [128, AM, DIMn], bf16, isOutput=False)
    WeT_d = nc.declare_dram_parameter("WeT", [128, DM, DIMn], bf16, isOutput=False)
    ba1_d = nc.declare_dram_parameter("ba1f", [128, AM], fp32, isOutput=False)
    be_d = nc.declare_dram_parameter("bef", [128, DM], fp32, isOutput=False)
    out_d = nc.declare_dram_parameter("out", [Bn, DIMn, NLOCn], fp32, isOutput=True)

    with tile.TileContext(nc) as tc:
        with (
            tc.tile_pool(name="wpool", bufs=1) as wpool,
            tc.tile_pool(name="bpool", bufs=2) as bpool,
            tc.tile_pool(name="dpool", bufs=1) as dpool,
            tc.tile_pool(name="kpool", bufs=2) as kpool,
            tc.tile_pool(name="kvpool", bufs=PF + 1) as kvpool,
            tc.tile_pool(name="cpool", bufs=2) as cpool,
            tc.tile_pool(name="c1pool", bufs=1) as c1pool,
            tc.tile_pool(name="ypool", bufs=2) as ypool,
            tc.tile_pool(name="pspool", bufs=4, space="PSUM") as pspool,
            tc.tile_pool(name="kqpool", bufs=1, space="PSUM") as kqpool,
            tc.tile_pool(name="vapool", bufs=1, space="PSUM") as vapool,
        ):
            # ---- load weights / constants once ----
            WkTn = wpool.tile([CINn, DIMn], bf16)
            WvT = wpool.tile([CINn, DIMn], bf16)
            WqTb = wpool.tile([4, DIMn], bf16)
            Wp1q = wpool.tile([4, PHIDn], bf16)
            Wp2T = wpool.tile([PHIDn + 1, DIMn], bf16)
            Wa1T = wpool.tile([128, KA1, AHIDn], bf16)
            Wa2T = wpool.tile([128, AM, DIMn], bf16)
            WeT = wpool.tile([128, DM, DIMn], bf16)
            ba1f = wpool.tile([128, AM], fp32)
            bef = wpool.tile([128, DM], fp32)
            for sb, dr in [(WkTn, WkTn_d), (WvT, WvT_d), (WqTb, WqTb_d),
                           (Wp1q, Wp1q_d), (Wp2T, Wp2T_d), (Wa1T, Wa1T_d),
                           (Wa2T, Wa2T_d), (WeT, WeT_d),
                           (ba1f, ba1_d), (bef, be_d)]:
                nc.sync.dma_start(out=sb[:], in_=dr[:])

            # packed distance array: hi u16 = fp16 dneg, lo u16 = iota
            dsb32 = wpool.tile([QT, Nn], u32, tag="dsb32")
            dsb_f32 = dsb32[:].bitcast(fp32)
            dsb_lo = dsb32[:].bitcast(u16).rearrange(
                "p (n two) -> p n two", two=2)
            dsb_hi = dsb32[:].bitcast(fp16).rearrange(
                "p (n two) -> p n two", two=2)
            def iota_fill():
                # one contiguous u32 load: lo halves = iota, hi halves = 0
                # (the hi halves are overwritten by the distance evictions
                # before any scan reads them)
                nc.sync.dma_start(out=dsb32[:], in_=iota_d[:])

            iota_fill()

            # pe1 activations with a trailing ones-row (bias row of Wp2T)
            pe1c = c1pool.tile([PHIDn + 1, CHUNK], bf16, tag="pe1c")
            nc.vector.memset(pe1c[PHIDn:PHIDn + 1, :], 1.0)

            prhs_sbs, plhs_sbs, pos16s = [], [], []
            for b in range(Bn):
                prhs_sb = bpool.tile([13, Nn], bf16, tag="prhs_sb")
                nc.sync.dma_start(out=prhs_sb[:], in_=paug_rhs[b])
                plhs_sb = bpool.tile([13, NLOCn], bf16, tag="plhs_sb")
                nc.sync.dma_start(out=plhs_sb[:], in_=paug_lhs[b])
                pos16 = bpool.tile([16, Nn], fp32, tag="pos16")
                nc.sync.dma_start(out=pos16[:], in_=pos16_d[b])
                prhs_sbs.append(prhs_sb); plhs_sbs.append(plhs_sb)
                pos16s.append(pos16)

            NCAND = NGRP * 8 if TWO_LEVEL else Nn

            def knn_start(b, t):
                """Emit-piece list for one KNN tile (distances + top-16)."""
                cand = kpool.tile([QT, max(NCAND, 16) if TWO_LEVEL else 8],
                                  fp32, tag="cand")
                v8a = kpool.tile([QT, 8], fp32, tag="v8a")
                v8b = kpool.tile([QT, 8], fp32, tag="v8b")
                idxg = kpool.tile([QT, 128], u16, tag="idxg")
                idxw16 = kpool.tile([128, QT], u16, tag="idxw16")

                def p_d(lo, hi):
                    for nch in range(lo, hi):
                        dps = pspool.tile([128, 512], fp32, tag="ps")
                        nc.tensor.matmul(
                            dps[0:QT, :], plhs_sbs[b][:, ts(t, QT)],
                            prhs_sbs[b][:, ts(nch, 512)])
                        # fp16 eviction into the u32 HIGH halves
                        nc.scalar.activation(
                            dsb_hi[:, nch * 512:(nch + 1) * 512, 1:2],
                            dps[0:QT, :], AF.Copy)

                def ext(v8, lohi):
                    # low u16 of each packed fp32 = original column index
                    nc.vector.tensor_copy(
                        idxg[:, lohi * 8:(lohi + 1) * 8],
                        v8[:].bitcast(u16).rearrange(
                            "p (k two) -> p k two", two=2)[:, :, 0:1])

                if TWO_LEVEL:
                    def p_l1(lo, hi):
                        for g in range(lo, hi):
                            nc.vector.max(
                                out=cand[:, g * 8:(g + 1) * 8],
                                in_=dsb_f32[:, g * GRP:(g + 1) * GRP])

                    def p_l2():
                        nc.vector.memset(idxg[:, 16:128], 0)
                        nc.vector.max(out=v8a[:], in_=cand[:, 0:NCAND])
                        ext(v8a, 0)
                        nc.vector.match_replace(
                            out=cand[:, 0:NCAND], in_to_replace=v8a[:],
                            in_values=cand[:, 0:NCAND], imm_value=NEG_BIG)
                        nc.vector.max(out=v8b[:], in_=cand[:, 0:NCAND])
                        ext(v8b, 1)
                        # transpose triggered from the ACT queue (HWDGE):
                        # keeps Sync's long semaphore waits out of the path
                        nc.scalar.dma_start(out=idxw16[:], in_=idxg[:],
                                            transpose=True)

                    # all KNN work done by chunk 2 of the carrier tile — the
                    # boundary chunk (c=3) gets a clean DVE queue and the idx
                    # transpose lands a full chunk before the gather needs it
                    pieces = [lambda: p_d(0, NCH_D),
                              lambda: p_l1(0, NGRP // 2),
                              lambda: (p_l1(NGRP // 2, NGRP), p_l2())]
                else:
                    def p_small():
                        nc.vector.memset(idxg[:, 16:128], 0)
                        nc.vector.max(out=v8a[:], in_=dsb_f32[:])
                        ext(v8a, 0)
                        nc.vector.match_replace(
                            out=dsb_f32[:], in_to_replace=v8a[:],
                            in_values=dsb_f32[:], imm_value=NEG_BIG)
                        nc.vector.max(out=v8b[:], in_=dsb_f32[:])
                        ext(v8b, 1)
                        nc.sync.dma_start(out=idxw16[:], in_=idxg[:],
                                          transpose=True)
                        # restore the iota lows that match_replace clobbered
                        iota_fill()

                    pieces = [lambda: p_d(0, NCH_D), p_small]

                return idxw16, pieces

            # ---- k/v chunk prefetch (rolling, PF chunks ahead) ----
            NGC = Bn * NQT * NCH_TILE          # global chunk count
            kv_bufs = {}

            def kv_prefetch(g):
                if g >= NGC:
                    return
                bb = g // (NQT * NCH_TILE)
                cc = g % (NQT * NCH_TILE)
                col0 = cc * CHUNK
                kbf = kvpool.tile([CINn, CHUNK], bf16, tag="kbf")
                vbf = kvpool.tile([CINn, CHUNK], bf16, tag="vbf")
                nc.gpsimd.dma_start(out=kbf[:],
                                    in_=key_r[bb, :, col0:col0 + CHUNK])
                nc.gpsimd.dma_start(out=vbf[:],
                                    in_=val_r[bb, :, col0:col0 + CHUNK])
                kv_bufs[g] = (kbf, vbf)

            # gather + cast prefetch (1 chunk ahead)
            pos_bufs = {}

            posg_bufs = {}

            def gather_only(g, idxw_for):
                if g >= NGC:
                    return
                bb = g // (NQT * NCH_TILE)
                cc = g % NCH_TILE
                posg = cpool.tile([16, CHUNK], fp32, tag="posg")
                nc.gpsimd.ap_gather(
                    posg[:], pos16s[bb][:],
                    idxw_for[0:16, cc * CQ:(cc + 1) * CQ].bitcast(i16),
                    channels=16, num_elems=Nn, d=1, num_idxs=CHUNK)
                posg_bufs[g] = posg

            def cast_only(g):
                if g >= NGC:
                    return
                posgb = cpool.tile([16, CHUNK], bf16, tag="posgb")
                nc.vector.tensor_copy(posgb[:], posg_bufs.pop(g)[:])
                pos_bufs[g] = posgb

            def gather_prefetch(g, idxw_for):
                gather_only(g, idxw_for)
                cast_only(g)

            tiles = [(b, t) for b in range(Bn) for t in range(NQT)]
            # prologue: KNN for tiles 0 and 1, k/v for chunks 0..PF-1
            for g in range(PF):
                kv_prefetch(g)
            idxw_list = [None] * len(tiles)
            idxw_list[0], pieces0 = knn_start(*tiles[0])
            for p in pieces0:
                p()
            if len(tiles) > 1:
                idxw_list[1], pieces = knn_start(*tiles[1])
            else:
                pieces = []
            gather_prefetch(0, idxw_list[0])

            pending_tail = [None]      # softmax tail closures from chunk c-1
            pending_tail_b = [None, None]

            for ti, (b, t) in enumerate(tiles):
                if t == 0:
                    aggsb = bpool.tile([128, DM, NLOCn], bf16, tag="aggsb")
                piece_i = [0]

                def run_piece(n=1):
                    stop = min(len(pieces), piece_i[0] + n)
                    while piece_i[0] < stop:
                        pieces[piece_i[0]]()
                        piece_i[0] += 1

                for c in range(NCH_TILE):
                    gc = ti * NCH_TILE + c
                    if c + 1 < NCH_TILE:
                        gather_only(gc + 1, idxw_list[ti])
                    elif ti + 1 < len(tiles):
                        # boundary gather uses the next tile's idx: make sure
                        # every remaining KNN piece (incl. its transpose) is
                        # emitted first (no-op for the 4-chunk full config)
                        run_piece(len(pieces))
                        gather_only(gc + 1, idxw_list[ti + 1])
                    posgb = pos_bufs.pop(gc)
                    kbf, vbf = kv_bufs.pop(gc)

                    # ---- pe1 = relu((Wp1f Wq) pos + b) straight from pos ----
                    p1ps = pspool.tile([128, 512], fp32, tag="ps")
                    nc.tensor.matmul(p1ps[0:PHIDn, :], Wp1q[:], posgb[0:4, :])
                    nc.scalar.activation(pe1c[0:PHIDn, :], p1ps[0:PHIDn, :],
                                         AF.Relu)
                    # ---- pe = Wp2 pe1 + bp2 (bias row rides the matmul) ----
                    peg = cpool.tile([128, DM, CHUNK], bf16, tag="peg")
                    for m in range(DM):
                        p2ps = pspool.tile([128, 512], fp32, tag="ps")
                        nc.tensor.matmul(p2ps[:], Wp2T[:, ts(m, 128)], pe1c[:])
                        nc.scalar.activation(peg[:, m, :], p2ps[:], AF.Copy)

                    # ---- rr = q - k_f + 1 accumulated on the PE ----
                    kqps = kqpool.tile([128, DM, CHUNK], fp32, tag="kq")
                    for m in range(DM):
                        nc.tensor.matmul(kqps[:, m, :], WqTb[:, ts(m, 128)],
                                         posgb[0:4, :], start=True, stop=False)
                        nc.tensor.matmul(kqps[:, m, :], WkTn[:, ts(m, 128)],
                                         kbf[:], start=False, stop=True)
                    a1in = cpool.tile([128, DM, CHUNK], bf16, tag="a1in")
                    nc.vector.tensor_mul(a1in[:], kqps[:], peg[:])

                    # ---- vpe = v + pe accumulated on the PE (the Wp2
                    # matmul is re-run into the v PSUM; bp2 rides the
                    # ones-row), evicted with one fused ACT copy ----
                    vps = vapool.tile([128, DM, CHUNK], fp32, tag="va")
                    for m in range(DM):
                        nc.tensor.matmul(vps[:, m, :], WvT[:, ts(m, 128)],
                                         vbf[:], start=True, stop=False)
                        nc.tensor.matmul(vps[:, m, :], Wp2T[:, ts(m, 128)],
                                         pe1c[:], start=False, stop=True)
                    vpe = cpool.tile([128, DM, CHUNK], bf16, tag="vpe")
                    nc.scalar.activation(vpe[:], vps[:], AF.Copy)

                    # softmax tail (part A) of the previous chunk: fills the
                    # DVE while this chunk's a1 matmuls run, but before the
                    # a1r evictions so those land just-in-time for a2
                    if pending_tail[0] is not None:
                        pending_tail[0]()
                        pending_tail[0] = None

                    # ---- a-branch MLP ----
                    a1r = cpool.tile([128, AM, CHUNK], bf16, tag="a1r")
                    for mt in range(AM):
                        a1ps = pspool.tile([128, CHUNK], fp32, tag="ps")
                        for kt in range(KA1):
                            nc.tensor.matmul(
                                a1ps[:], Wa1T[:, kt, ts(mt, 128)],
                                a1in[:, kt, :],
                                start=(kt == 0), stop=(kt == KA1 - 1))
                        if mt % 8 in (0, 1, 3, 5, 7):
                            nc.scalar.activation(a1r[:, mt, :], a1ps[:],
                                                 AF.Relu,
                                                 bias=ba1f[:, mt:mt + 1])
                        else:
                            nc.vector.tensor_scalar(
                                a1r[:, mt, :], a1ps[:],
                                ba1f[:, mt:mt + 1], 0.0,
                                op0=OP.add, op1=OP.max)
                    cast_only(gc + 1)
                    a2ps = vapool.tile([128, DM, CHUNK], fp32, tag="va")
                    for m in range(DM):
                        for kt in range(AM):
                            nc.tensor.matmul(
                                a2ps[:, m, :], Wa2T[:, kt, ts(m, 128)],
                                a1r[:, kt, :],
                                start=(kt == 0), stop=(kt == AM - 1))
                    ee = cpool.tile([128, DM, CHUNK], bf16, tag="ee")
                    nc.scalar.activation(ee[:], a2ps[:], AF.Exp)

                    # end-of-chunk: KNN piece for the next tile (its idx
                    # transpose must be emitted before any gather using it),
                    # then next-chunk gather + k/v prefetch.  These sit at
                    # the TAIL of the DVE/gpsimd queues so they never
                    # head-block a1in/vpe/a1r that the PE is waiting on.
                    if c < NCH_TILE - 1:
                        run_piece(1)
                    kv_prefetch(gc + PF)

                    # softmax tail (part B), skewed TWO chunks back: pure
                    # slack — feeds only aggsb/final conv — so it never sits
                    # ahead of the next chunk's a1in in the DVE queue
                    if pending_tail_b[1] is not None:
                        pending_tail_b[1]()
                    pending_tail_b[1] = pending_tail_b[0]
                    pending_tail_b[0] = None

                    erec = cpool.tile([128, DM, CQ], fp32, tag="erec")
                    evpe = cpool.tile([128, DM, CHUNK], bf16, tag="evpe")
                    sums = cpool.tile([128, DM, 2, CQ], bf16, tag="sums")
                    col_lo = t * QT + c * CQ
                    ee_l, vpe_l = ee, vpe

                    def tail_a(ee=ee_l, vpe=vpe_l, evpe=evpe):
                        nc.vector.tensor_mul(evpe[:], ee[:], vpe[:])

                    def tail_b(ee=ee_l, sums=sums, erec=erec, evpe=evpe,
                               col_lo=col_lo, aggsb=aggsb):
                        # one pass: group-sums of ee (softmax denom) and of
                        # ee*vpe (numerator) — ee and evpe are consecutive
                        # ring tags is not guaranteed, so reduce via a 5D AP
                        # over each separately is replaced by two slices of
                        # one output tile
                        with nc.allow_low_precision("16-way softmax sums"):
                            nc.vector.tensor_reduce(
                                sums[:, :, 0, :],
                                ee[:].rearrange("p m (g k) -> p m g k", k=KKn),
                                axis=AX.X, op=OP.add)
                            nc.vector.tensor_reduce(
                                sums[:, :, 1, :],
                                evpe[:].rearrange("p m (g k) -> p m g k", k=KKn),
                                axis=AX.X, op=OP.add)
                        nc.vector.reciprocal(erec[:], sums[:, :, 0, :])
                        nc.vector.tensor_mul(
                            aggsb[:, :, col_lo:col_lo + CQ],
                            sums[:, :, 1, :], erec[:])

                    pending_tail[0] = tail_a
                    pending_tail_b[0] = tail_b

                for p in pieces[NCH_TILE:]:
                    p()
                # kick the KNN pipeline for tile ti+2
                if ti + 2 < len(tiles):
                    idxw_list[ti + 2], pieces = knn_start(*tiles[ti + 2])
                else:
                    pieces = []

                # ---- final 1x1 conv once this batch's tiles are done ----
                if t == NQT - 1:
                    if pending_tail[0] is not None:
                        pending_tail[0]()
                        pending_tail[0] = None
                    if pending_tail_b[1] is not None:
                        pending_tail_b[1]()
                        pending_tail_b[1] = None
                    if pending_tail_b[0] is not None:
                        pending_tail_b[0]()
                        pending_tail_b[0] = None
                    for nloc0 in range(0, NLOCn, 512):
                        w = min(512, NLOCn - nloc0)
                        for m in range(DM):
                            yps = pspool.tile([128, 512], fp32, tag="ps")
                            for kt in range(DM):
                                nc.tensor.matmul(
                                    yps[:, :w], WeT[:, kt, ts(m, 128)],
                                    aggsb[:, kt, nloc0:nloc0 + w],
                                    start=(kt == 0), stop=(kt == DM - 1))
                            yev = ypool.tile([128, 512], fp32, tag="yev")
                            nc.vector.tensor_scalar_add(yev[:, :w], yps[:, :w],
                                                        bef[:, m:m + 1])
                            nc.sync.dma_start(
                                out=out_d[b, ts(m, 128), nloc0:nloc0 + w],
                                in_=yev[:, :w])

    nc.finalize()   # Bacc.finalize: wait legalization, library loads, ISA codegen
    return nc


def host_prepare(inputs, dims, ncores=NCORES):
    """Fold BN/biases into weights, pre-transpose for the PE, shard by n."""
    d = dims
    f32 = np.float32
    key = np.asarray(inputs["key"], f32)
    values = np.asarray(inputs["values"], f32)
    pos = np.asarray(inputs["pos"], f32)
    g = lambda n: np.asarray(inputs[n], f32)

    Wk, bk = g("Wk"), g("bk")
    Wq, bq = g("Wq"), g("bq")
    Wv, bv = g("Wv"), g("bv")
    Wp1, bp1 = g("Wp1"), g("bp1")
    Wp2, bp2 = g("Wp2"), g("bp2")
    Wa1, ba1 = g("Wa1"), g("ba1")
    Wa2 = g("Wa2")
    We, be = g("We"), g("be")

    p_sc = g("p_gamma") / np.sqrt(g("p_var") + f32(BN_EPS))
    Wp1f = (Wp1 * p_sc[:, None]).astype(f32)
    bp1f = (bp1 * p_sc + g("p_beta") - g("p_mean") * p_sc).astype(f32)
    a_sc = g("a_gamma") / np.sqrt(g("a_var") + f32(BN_EPS))
    Wa1f = (Wa1 * a_sc[:, None]).astype(f32)
    ba1f = (ba1 * a_sc + g("a_beta") - g("a_mean") * a_sc).astype(f32)

    DM = d["DIM"] // 128
    AM = d["AHID"] // 128
    KA1 = d["DIM"] // 128
    QT = min(128, d["NLOC"])

    def colsplit(v, nt):  # (nt*128,) -> (128, nt)
        return np.ascontiguousarray(v.reshape(nt, 128).T).astype(f32)

    # pe1 = relu(Wp1f q + bp1f) with q = Wq pos + bq folds to a 4-row conv
    Wp1q = np.concatenate(
        [(Wp1f @ Wq).T, (Wp1f @ bq + bp1f)[None, :]], 0)          # (4, PHID)
    Wp2Tb = np.concatenate([Wp2.T, bp2[None, :]], 0)              # (PHID+1, DIM)

    common = {
        "WkTn": np.ascontiguousarray(-Wk.T).astype(BF16),
        "WvT": np.ascontiguousarray(Wv.T).astype(BF16),
        "WqTb": np.ascontiguousarray(np.concatenate(
            [Wq.T, (bq - bk + 1.0)[None, :]], 0)).astype(BF16),
        "Wp1q": np.ascontiguousarray(Wp1q).astype(BF16),
        "Wp2T": np.ascontiguousarray(Wp2Tb).astype(BF16),
        "Wa1T": np.ascontiguousarray(
            Wa1f.T.reshape(KA1, 128, d["AHID"]).transpose(1, 0, 2)).astype(BF16),
        "Wa2T": np.ascontiguousarray(
            Wa2.T.reshape(AM, 128, d["DIM"]).transpose(1, 0, 2)).astype(BF16),
        "WeT": np.ascontiguousarray(
            We.T.reshape(DM, 128, d["DIM"]).transpose(1, 0, 2)).astype(BF16),
        "ba1f": colsplit(ba1f, AM),
        "bef": colsplit((We @ bv + be).astype(f32), DM),
        "iota32": np.ascontiguousarray(
            np.broadcast_to(np.arange(d["N"], dtype=np.uint32)[None, :],
                            (QT, d["N"]))),
    }

    # hi/lo bf16 split of pos, |p_j|^2 and |p_i|^2 for the centered distance
    # matmul: dneg ~= -d  (top values near 0 -> fp16 eviction is precise)
    sq = (pos * pos).sum(axis=1).astype(f32)              # (B, N)
    pos_hi = pos.astype(BF16)
    pos_lo = (pos - pos_hi.astype(f32)).astype(BF16)
    sq_hi = sq.astype(BF16)
    sq_lo = (sq - sq_hi.astype(f32)).astype(BF16)
    paug_rhs = np.concatenate(
        [2.0 * pos_hi.astype(f32), 2.0 * pos_lo.astype(f32),
         2.0 * pos_hi.astype(f32), -sq_hi.astype(f32)[:, None, :],
         -sq_lo.astype(f32)[:, None, :],
         -np.ones((d["B"], 2, d["N"]), f32)], 1).astype(BF16)
    pos16 = np.zeros((d["B"], 16, d["N"]), f32)
    pos16[:, 0:3] = pos
    pos16[:, 3] = 1.0

    in_maps = []
    for cid in range(ncores):
        n0 = cid * d["NLOC"]
        n1 = n0 + d["NLOC"]
        m = dict(common)
        m["key_r"] = np.ascontiguousarray(key[:, :, n0:n1, :]).reshape(
            d["B"], d["CIN"], d["NLOC"] * d["KK"]).astype(BF16)
        m["val_r"] = np.ascontiguousarray(values[:, :, n0:n1, :]).reshape(
            d["B"], d["CIN"], d["NLOC"] * d["KK"]).astype(BF16)
        m["paug_lhs"] = np.ascontiguousarray(np.concatenate(
            [pos_hi.astype(f32)[:, :, n0:n1], pos_hi.astype(f32)[:, :, n0:n1],
             pos_lo.astype(f32)[:, :, n0:n1],
             np.ones((d["B"], 2, d["NLOC"]), f32),
             sq_hi.astype(f32)[:, None, n0:n1],
             sq_lo.astype(f32)[:, None, n0:n1]], 1)).astype(BF16)
        m["paug_rhs"] = paug_rhs
        m["pos16"] = pos16
        in_maps.append(m)
    return in_maps


_NC_CACHE = {}


def _get_nc(dims_key):
    if dims_key not in _NC_CACHE:
        _NC_CACHE[dims_key] = build_nc(_dims_full())
    return _NC_CACHE[dims_key]


def kernel(**inputs):
    from concourse.bass_utils import run_bass_kernel_spmd
    dims = _dims_full()
    nc = _get_nc("full")
    in_maps = host_prepare(inputs, dims)
    res = run_bass_kernel_spmd(nc, in_maps, core_ids=list(range(NCORES)))
    outs = [r["out"].astype(np.float32) for r in res.results]
    return np.concatenate(outs, axis=2)

